# revision 36
# baseline (speedup 1.0000x reference)
"""Paged GQA decode attention (sparse_attention) on 8 TRN2 NeuronCores.

Sharding: data-parallel over the 16 sequences -- each core owns 2 sequences
and their full KV gather (4KB rows, best DMA efficiency), zero collectives.

Per core pipeline (all sizes hardcoded for the graded problem):
  - scatter new k/v rows into this core's private cache copy (indirect DMA)
  - per sequence, per 512-slot chunk: dma_gather K rows -> PE-transpose per
    head -> QK^T (Q^T stationary, f32r moving) -> exp via ScalarE with fused
    row-sum accumulation -> PE-transpose P -> PV accumulation in PSUM
  - normalize by reciprocal row sums, DMA out.
"""

import ml_dtypes
import numpy as np

# ---- problem constants (must match the harness's reference.py) ----
NUM_HEADS = 32
NUM_KV_HEADS = 8
HEAD_DIM = 128
BS = 16
KV_LEN = 2048
NUM_SLOTS = BS * KV_LEN          # 32768
D = NUM_KV_HEADS * HEAD_DIM      # 1024 (cache row width, f32)
SCALE = HEAD_DIM ** -0.5
N_CORES = 8
GROUP = NUM_HEADS // NUM_KV_HEADS  # 4


class Cfg:
    """Build-time sizes. Defaults = the graded problem; smaller variants are
    used by the dev-only simulator tests."""

    def __init__(self, bs=BS, kv_len=KV_LEN, num_slots=NUM_SLOTS,
                 n_cores=N_CORES, ch_tiles=4):
        self.bs = bs
        self.kv_len = kv_len
        self.num_slots = num_slots
        self.n_cores = n_cores
        self.seq_per_core = bs // n_cores
        self.ch_tiles = ch_tiles                 # 128-slot tiles per chunk
        self.ch_slots = 128 * ch_tiles           # gather granularity
        assert kv_len % self.ch_slots == 0
        self.nchunk = kv_len // self.ch_slots
        ntiles = kv_len // 128
        if ntiles > 4:
            # taper the schedule so the last chunk's post-DMA compute is small
            self.chunks = [4] * (ntiles // 4 - 1) + [3, 1]
        else:
            self.chunks = [ntiles]
        assert sum(self.chunks) == ntiles
        self.q_cols = self.seq_per_core * NUM_HEADS  # qT columns


CFG = Cfg()


def build_program(cfg=CFG, dep_mask=None):
    # dep_mask: set of (seq_local, chunk) whose gathers must wait for the
    # kv scatter (host-computed from the actual indices; None = all)

    import concourse.bacc as bacc
    import concourse.bass as bass
    import concourse.mybir as mybir
    import concourse.tile as tile
    from concourse.tile_rust import add_dep_helper

    f32 = mybir.dt.float32
    f32r = mybir.dt.float32r
    bf16 = mybir.dt.bfloat16
    i32 = mybir.dt.int32
    i16 = mybir.dt.int16
    EXP = mybir.ActivationFunctionType.Exp
    MULT = mybir.AluOpType.mult
    X = mybir.AxisListType.X

    S = cfg.seq_per_core
    CHUNKS = cfg.chunks
    NC_ = len(CHUNKS)
    TT = sum(CHUNKS)              # total 128-slot tiles per sequence
    ICOLS = cfg.kv_len // 16      # idx columns per sequence

    nc = bacc.Bacc("TRN2", target_bir_lowering=False, debug=False,
                   enable_asserts=False, num_devices=cfg.n_cores,
                   num_swdge_queues=1)

    kc = nc.dram_tensor("k_cache", [cfg.num_slots, D], f32r, kind="ExternalInput").ap()
    vc = nc.dram_tensor("v_cache", [cfg.num_slots, D], f32r, kind="ExternalInput").ap()
    knew_d = nc.dram_tensor("k_new", [cfg.bs, D], f32r, kind="ExternalInput").ap()
    vnew_d = nc.dram_tensor("v_new", [cfg.bs, D], f32r, kind="ExternalInput").ap()
    slot_d = nc.dram_tensor("slot_idx", [cfg.bs, 1], i32, kind="ExternalInput").ap()
    qT_d = nc.dram_tensor("qT", [HEAD_DIM, cfg.q_cols], bf16, kind="ExternalInput").ap()
    pi_d = nc.dram_tensor("pi16", [128, S * ICOLS], i16, kind="ExternalInput").ap()
    ident_d = nc.dram_tensor("ident", [128, 128], f32r, kind="ExternalInput").ap()
    out_d = nc.dram_tensor("out", [S, NUM_HEADS * HEAD_DIM], f32, kind="ExternalOutput").ap()

    with tile.TileContext(nc) as tc:
        with tc.tile_pool(name="const", bufs=1) as constp, \
             tc.tile_pool(name="kv", bufs=3) as kvp, \
             tc.tile_pool(name="kt", bufs=3) as ktp, \
             tc.tile_pool(name="exps", bufs=3) as expp, \
             tc.tile_pool(name="misc", bufs=2) as miscp, \
             tc.tile_pool(name="ps_kt", bufs=2, space="PSUM") as ps_kt, \
             tc.tile_pool(name="ps_s", bufs=2, space="PSUM") as ps_s, \
             tc.tile_pool(name="ps_sum", bufs=2, space="PSUM") as ps_sum, \
             tc.tile_pool(name="ps_pv", bufs=2, space="PSUM") as ps_pv:

            # index load first (gates the first gather); scatter inputs go
            # on the scalar HWDGE queue in parallel
            idx_sb = constp.tile([128, S * ICOLS], i16)
            nc.sync.dma_start(idx_sb[:], pi_d)
            slot_sb = constp.tile([cfg.bs, 1], i32)
            nc.scalar.dma_start(slot_sb[:], slot_d)
            knew_sb = constp.tile([cfg.bs, D], f32r)
            nc.scalar.dma_start(knew_sb[:], knew_d)
            vnew_sb = constp.tile([cfg.bs, D], f32r)
            nc.scalar.dma_start(vnew_sb[:], vnew_d)
            qt_sb = constp.tile([128, cfg.q_cols], bf16)
            nc.sync.dma_start(qt_sb[:], qT_d)
            ident = constp.tile([128, 128], f32r)
            nc.sync.dma_start(ident[:], ident_d)
            ones_f = constp.tile([128, 2], f32)
            nc.vector.memset(ones_f[:], 1.0)
            ones = constp.tile([128, 2], f32r)
            nc.vector.tensor_copy(ones[:], ones_f[:])

            # warm-up gather: loads the gather ucode library + descriptor
            # rings while the input DMAs are still in flight (result unused)
            warm_idx = constp.tile([128, 1], i16)
            nc.gpsimd.memset(warm_idx[:], 0)
            warm_dst = constp.tile([128, D], f32r)
            n_regs = {n: nc.gpsimd.to_reg(n * 128) for n in sorted(set(CHUNKS))}
            nc.gpsimd.dma_gather(warm_dst[:].rearrange("p (o e) -> p o e", o=1),
                                 kc, warm_idx[:], 16, 16, D, elem_step=D)

            sc_k = sc_v = None
            # emit the cache scatter right before the first gather that needs
            # it (so waiting on the k/v input DMAs never stalls the stream)
            if dep_mask is None:
                scatter_at = 0
            else:
                scatter_at = min((bb * NC_ + cc for (bb, cc) in dep_mask),
                                 default=1)
            out_v = out_d.rearrange("b (x d) -> (b x) d", d=HEAD_DIM)

            for b in range(S):
                # fp32r matmuls must write at PSUM partition 0, so the two
                # head-sets accumulate into two separate banks
                pv0 = ps_pv.tile([128, 512], f32, tag="pv", name="pv0")
                pv1 = ps_pv.tile([128, 512], f32, tag="pv", name="pv1")
                pvs = [pv0, pv1]
                sum0 = ps_sum.tile([128, 2], f32, tag="sum", name="sum0")
                sum1 = ps_sum.tile([128, 2], f32, tag="sum", name="sum1")
                sums = [sum0, sum1]

                toff = 0
                for c in range(NC_):
                    CT = CHUNKS[c]
                    CS = CT * 128
                    if sc_k is None and b * NC_ + c >= scatter_at:
                        sc_k = nc.gpsimd.indirect_dma_start(
                            out=kc, in_=knew_sb[:],
                            out_offset=bass.IndirectOffsetOnAxis(
                                ap=slot_sb[:, :1], axis=0),
                            in_offset=None)
                        sc_v = nc.gpsimd.indirect_dma_start(
                            out=vc, in_=vnew_sb[:],
                            out_offset=bass.IndirectOffsetOnAxis(
                                ap=slot_sb[:, :1], axis=0),
                            in_offset=None)
                    icol0 = b * ICOLS + toff * 8
                    idx_ap = idx_sb[:, icol0:icol0 + CT * 8]

                    knat = kvp.tile([128, CT, D], f32r, tag="knat")
                    g1 = nc.gpsimd.dma_gather(knat[:], kc, idx_ap, CS,
                                              n_regs[CT], D, elem_step=D)
                    vnat = kvp.tile([128, CT, D], f32r, tag="vnat")
                    g2 = nc.gpsimd.dma_gather(vnat[:], vc, idx_ap, CS,
                                              n_regs[CT], D, elem_step=D)
                    if dep_mask is None or (b, c) in dep_mask:
                        add_dep_helper(g1.ins, sc_k.ins, reason="scatter before gather")
                        add_dep_helper(g2.ins, sc_v.ins, reason="scatter before gather")

                    # K^T: PE transposes (4 per PSUM bank) + bulk PSUM->SBUF
                    # copies (rounding f32 -> f32r for the QK stationary)
                    ktsb = ktp.tile([128, NUM_KV_HEADS, CS], bf16, tag="ktsb")
                    for t in range(CT):
                        for hg in range(2):
                            ktps = ps_kt.tile([128, 512], f32r, tag="ktps")
                            for i in range(4):
                                h = hg * 4 + i
                                nc.tensor.transpose(
                                    ktps[:, i * 128:(i + 1) * 128],
                                    knat[:, t, h * 128:(h + 1) * 128],
                                    ident[:])
                            dst = ktsb[:, hg * 4:hg * 4 + 4, t * 128:t * 128 + 128]
                            src = ktps[:].rearrange("p (i d) -> p i d", d=128)
                            if (t * 2 + hg) % 2 == 0:
                                nc.vector.tensor_copy(dst, src)
                            else:
                                nc.scalar.copy(dst, src)

                    # S^T = K @ Q^T per (tile, head): [128 slots, 4] blocks at
                    # partition 0; one [128, CT*32] PSUM tile per chunk
                    st_ps = ps_s.tile([128, CT * 32], f32, tag="stps")
                    for t in range(CT):
                        for h in range(NUM_KV_HEADS):
                            qcol = (b * NUM_KV_HEADS + h) * GROUP
                            nc.tensor.matmul(
                                out=st_ps[:, t * 32 + h * GROUP:
                                          t * 32 + h * GROUP + GROUP],
                                lhsT=ktsb[:, h, t * 128:(t + 1) * 128],
                                rhs=qt_sb[:, qcol:qcol + GROUP],
                                start=True, stop=True)

                    # exp(S^T * scale) -> P^T directly (slots on partitions)
                    expsb = expp.tile([128, CT, 32], f32r, tag="exps")
                    nc.scalar.activation(
                        expsb[:].rearrange("p t x -> p (t x)"), st_ps[:],
                        EXP, scale=SCALE)

                    for t in range(CT):
                        gt = toff + t
                        # PV per head-set: 16-col P^T slice -> rows 0-15
                        for st in range(2):
                            nc.tensor.matmul(
                                out=pvs[st][0:16, :],
                                lhsT=expsb[:, t, 16 * st:16 * st + 16],
                                rhs=vnat[:, t, st * 512:(st + 1) * 512],
                                start=(gt == 0), stop=(gt == TT - 1))
                            # softmax row sums via two ones columns (even N);
                            # each set gets its own bank (one accumulation
                            # group per PSUM zero region)
                            nc.tensor.matmul(
                                out=sums[st][0:16, :],
                                lhsT=expsb[:, t, 16 * st:16 * st + 16],
                                rhs=ones[:],
                                start=(gt == 0), stop=(gt == TT - 1))

                    toff += CT

                # normalize: o = pv / rowsum; strips at 32-aligned SBUF bases
                recip = miscp.tile([64, 1], f32, tag="recip")
                nc.vector.reciprocal(recip[0:16, :], sums[0][0:16, 0:1])
                nc.vector.reciprocal(recip[32:48, :], sums[1][0:16, 0:1])
                o_stage = miscp.tile([64, 512], f32, tag="ostage")
                for st in range(2):
                    nc.vector.tensor_scalar(
                        out=o_stage[32 * st:32 * st + 16, :],
                        in0=pvs[st][0:16, :],
                        scalar1=recip[32 * st:32 * st + 16, :],
                        scalar2=None, op0=MULT)
                # final assembly: per-head diagonal blocks to DRAM
                for st in range(2):
                    for a in range(4):
                        h = st * 4 + a
                        eng = nc.sync if (a % 2 == 0) else nc.scalar
                        eng.dma_start(
                            out_v[b * NUM_HEADS + h * GROUP:
                                  b * NUM_HEADS + h * GROUP + GROUP, :],
                            o_stage[32 * st + 4 * a:32 * st + 4 * a + 4,
                                    128 * a:128 * a + 128])

    nc.compile()
    return nc


def shard_inputs(q, k, v, k_cache, v_cache, slot_mapping, page_indices, cfg=CFG):
    """Build per-core input maps (host-side sharding / index re-layout only)."""
    S = cfg.seq_per_core
    ICOLS = cfg.kv_len // 16
    q = np.ascontiguousarray(np.asarray(q, dtype=np.float32))
    k = np.ascontiguousarray(np.asarray(k, dtype=np.float32))
    v = np.ascontiguousarray(np.asarray(v, dtype=np.float32))
    k_cache = np.ascontiguousarray(np.asarray(k_cache, dtype=np.float32))
    v_cache = np.ascontiguousarray(np.asarray(v_cache, dtype=np.float32))
    slot_mapping = np.asarray(slot_mapping, dtype=np.int32).reshape(cfg.bs, 1)
    page_indices = np.asarray(page_indices, dtype=np.int32)

    in_maps = []
    for i in range(cfg.n_cores):
        sl = slice(i * S, (i + 1) * S)
        qc = q[sl].reshape(S, NUM_HEADS, HEAD_DIM)
        qT = np.ascontiguousarray(
            qc.transpose(2, 0, 1).reshape(HEAD_DIM, cfg.q_cols)
        ).astype(ml_dtypes.bfloat16)
        # dma_gather index layout: index j of sequence b lives at
        # [partition j%16, column b*ICOLS + j//16]
        pi_c = page_indices[sl]                       # [S, kv_len]
        w = pi_c.reshape(S, ICOLS, 16).transpose(2, 0, 1)   # [16, S, ICOLS]
        # the gather ucode fans descriptor generation across 8 Q7 cores, each
        # reading its own 16-partition replica of the index tile
        idx16 = np.ascontiguousarray(np.tile(
            w.reshape(16, S * ICOLS).astype(np.int16), (8, 1)))
        in_maps.append({
            "k_cache": k_cache,
            "v_cache": v_cache,
            "k_new": k,
            "v_new": v,
            "slot_idx": slot_mapping,
            "qT": qT,
            "pi16": idx16,
            "ident": np.eye(128, dtype=np.float32),
        })

    # which (local seq, chunk) gathers read a slot the scatter writes --
    # union over cores so all cores share one program
    dep_mask = set()
    ss = set(int(x) for x in slot_mapping.ravel())
    bounds = np.cumsum([0] + [n * 128 for n in cfg.chunks])
    for i in range(cfg.n_cores):
        for bl in range(S):
            row = page_indices[i * S + bl]
            for c in range(len(cfg.chunks)):
                if any(int(x) in ss for x in row[bounds[c]:bounds[c + 1]]):
                    dep_mask.add((bl, c))
    return in_maps, dep_mask


_PROGS = {}
last_results = None  # BassKernelResults of the most recent kernel() call


def kernel(q, k, v, k_cache, v_cache, slot_mapping, page_indices):
    global last_results
    from concourse.bass_utils import run_bass_kernel_spmd

    in_maps, dep_mask = shard_inputs(q, k, v, k_cache, v_cache, slot_mapping,
                                     page_indices, CFG)
    key = frozenset(dep_mask)
    if key not in _PROGS:
        _PROGS[key] = build_program(CFG, dep_mask)
    res = run_bass_kernel_spmd(_PROGS[key], in_maps,
                               core_ids=list(range(CFG.n_cores)))
    last_results = res
    out = np.concatenate([res.results[i]["out"] for i in range(CFG.n_cores)],
                         axis=0)
    return out
